# revision 16
# baseline (speedup 1.0000x reference)
"""DBA solver kernel for Trainium2 (8 NeuronCores, batch-parallel).

Host precomputes (numpy, per node):
  conf=w0, nl=w1; H_pd[p] = sum_c conf*jp[c,p]*jd[c]; H_dd = sum_c conf*jd^2
  g_d = sum_c conf*jd*r;  s = 1/max(H_dd + lam + nl + 1e-4, 1e-4)
  Z = [sqrt(conf)*jp0(6), sqrt(conf)*r0, sqrt(conf)*jp1(6), sqrt(conf)*r1,
       sqrt(s)*H_pd(6), sqrt(s)*g_d]  (21 cols),  col 21 = sqrt(s).

Device per core (2 batches), node n = p*NP + col:
  One self-Gram matmul per 128-node column accumulates G = sum Z^T Z (21x21):
    H_pp = G[0:6,0:6]+G[7:13,7:13]; g_p = G[0:6,6]+G[7:13,13]
    term = G[14:20,14:20];          g_corr = G[14:20,20]
  H_eff = H_pp + lam*I - term (+1e8 blk if iter>=2) + I; diag*=1.01
  g_eff = g_p - g_corr (rows 3: zeroed if iter>=2); 6x6 Gauss-Jordan solve.
  Depth pass from resident ZR = Z[:,14:22]:
    dd = clip(sqrt(s)*(ZR[6] - sum_p ZR[p]*dp[p]), -5, 5)
Host clips dp to [-2,2]; global-NaN -> zeros.
"""

import sys

import numpy as np

sys.path.insert(0, "/opt/trn_rl_repo")

import concourse.bass as bass  # noqa: E402
import concourse.tile as tile  # noqa: E402
from concourse import mybir  # noqa: E402
from concourse.bass_utils import run_bass_kernel_spmd  # noqa: E402

F32 = mybir.dt.float32
AL = mybir.AluOpType

# The walrus build in this container supports only ONE embedded sem-wait per
# instruction; Tile's kernel-tail drain aggregates one wait per engine/DMA
# lane (~10).  Split them across a chain of single-wait drains.
from concourse.vector_clock import ScopedClock  # noqa: E402


def _split_drain_and_barrier(self, tick_clock, wait_clock):
    drain_inst = self.nc.sync.drain()
    wait_clock.add_sem_waits(
        drain_inst.ins, ScopedClock({None: tick_clock.global_clock})
    )
    si = drain_inst.ins.sync_info
    waits = list(si.on_wait) if si and si.on_wait else []
    if len(waits) > 1:
        drain_inst.ins.sync_info = mybir.SyncInfo(
            on_wait=[waits[0]], on_update=list(si.on_update or [])
        )
        for wv in waits[1:]:
            d2 = self.nc.sync.drain()
            d2.ins.sync_info = mybir.SyncInfo(on_wait=[wv], on_update=[])
    self.nc.all_engine_barrier()
    popped = self.nc._tile_sem_poison_stack.pop()
    assert popped is self._sem_poison
    self.nc.clear_and_free_semaphores(list(self.sems.allocated().values()))
    self.nc.all_engine_barrier()


tile.TileContext._drain_and_barrier = _split_drain_and_barrier


def _legalize_waits(nc):
    """Walrus here allows one embedded wait per instruction; hoist extras
    onto preceding single-wait Drain instructions on the same engine."""
    for fn in nc.m.functions:
        for bb in fn.blocks:
            insts = list(bb.instructions)
            out = []
            changed = False
            for inst in insts:
                si = inst.sync_info
                if si is not None and si.on_wait and len(si.on_wait) > 1:
                    waits = list(si.on_wait)
                    for wv in waits[:-1]:
                        d = mybir.InstDrain(
                            name=nc.get_next_instruction_name(),
                            ins=[],
                            outs=[],
                            bass_is_fusable=False,
                        )
                        d.engine = inst.engine
                        d.sync_info = mybir.SyncInfo(on_wait=[wv], on_update=[])
                        try:
                            nc.register_instruction(d, overwrite=True)
                        except Exception:
                            pass
                        out.append(d)
                    inst.sync_info = mybir.SyncInfo(
                        on_wait=[waits[-1]], on_update=list(si.on_update or [])
                    )
                    changed = True
                out.append(inst)
            if changed:
                bb.instructions = out

NCORES = 8
BPC = 2
P = 128
W = 22  # record width: 21 Z cols + sqrt(s)
GD = 21  # gram dim


def _ap(base, dims, doff=0):
    """View on `base` keeping its partition dim, with explicit free dims."""
    return bass.AP(
        tensor=base.tensor,
        offset=base.offset + doff,
        ap=[list(base.ap[0])] + [list(d) for d in dims],
    )


def _dap(base, dims, doff=0):
    """DRAM-side DMA AP with fully explicit dims (first dim = partitions)."""
    return bass.AP(
        tensor=base.tensor, offset=base.offset + doff, ap=[list(d) for d in dims]
    )


def build_kernel(NP, T, iter_big):
    NT = NP // T
    nc = bass.Bass()

    X = nc.declare_dram_parameter("x", [BPC, P, NP, W], F32, isOutput=False)
    LAM = nc.declare_dram_parameter("lam", [BPC, 1], F32, isOutput=False)
    DD = nc.declare_dram_parameter("dd", [BPC, P, NP], F32, isOutput=True)
    DPO = nc.declare_dram_parameter("dp", [BPC, 6], F32, isOutput=True)

    with tile.TileContext(nc) as tc:
        with (
            tc.tile_pool(name="dram", bufs=1, space="DRAM") as dram_pool,
            tc.tile_pool(name="xf", bufs=4) as xf_pool,
            tc.tile_pool(name="res", bufs=1) as res_pool,
            tc.tile_pool(name="sm", bufs=1) as sm_pool,
            tc.tile_pool(name="ps", bufs=2, space="PSUM") as ps_pool,
        ):
            scr = dram_pool.tile([BPC, GD * GD], F32, tag="scr")
            dpb = dram_pool.tile([BPC, 6], F32, tag="dpb")
            zr = [res_pool.tile([P, NP, 8], F32, tag=f"zr{b}", name=f"zr{b}")
                  for b in range(BPC)]
            ddb = [res_pool.tile([P, NP], F32, tag=f"ddb{b}", name=f"ddb{b}")
                   for b in range(BPC)]
            psum = [ps_pool.tile([GD, GD], F32, tag=f"ps{b}", name=f"ps{b}")
                    for b in range(BPC)]

            for b in range(BPC):
                for t in range(NT):
                    xf = xf_pool.tile([P, T, W], F32, tag="xf")
                    nc.sync.dma_start(out=xf[:], in_=X[b, :, t * T : (t + 1) * T, :])
                    nc.vector.tensor_copy(
                        zr[b][:, t * T : (t + 1) * T, :], xf[:, :, 14:22]
                    )
                    for j in range(T):
                        z = xf[:, j, 0:GD]
                        nc.tensor.matmul(
                            psum[b][:],
                            z,
                            z,
                            start=(t == 0 and j == 0),
                            stop=(t == NT - 1 and j == T - 1),
                        )

            # ---- extract Gram -> DRAM -> per-batch-partition gather ----
            gsb = sm_pool.tile([GD, BPC, GD], F32, tag="gsb")
            for b in range(BPC):
                nc.vector.tensor_copy(gsb[:, b, :], psum[b][:])
            nc.sync.dma_start(
                out=_dap(scr[:], [[GD, GD], [GD * GD, BPC], [1, GD]]), in_=gsb[:]
            )
            gfull = sm_pool.tile([BPC, GD * GD], F32, tag="gfull")
            nc.sync.dma_start(out=gfull[:], in_=scr[:])
            lamt0 = sm_pool.tile([BPC, 1], F32, tag="lamt0")
            nc.sync.dma_start(out=lamt0[:], in_=LAM[:, 0:1])
            lamt = sm_pool.tile([BPC, 1], F32, tag="lamt")
            nc.vector.tensor_copy(lamt[:], lamt0[:])
            gb = gfull[:]

            aug = sm_pool.tile([BPC, 6, 7], F32, tag="aug")
            t66 = sm_pool.tile([BPC, 6, 6], F32, tag="t66")
            t6 = sm_pool.tile([BPC, 6], F32, tag="t6")
            ab = aug[:]
            # diag blocks at offsets 0,154,308; g cols at 6,160,314
            nc.vector.tensor_add(
                t66[:], _ap(gb, [[GD, 6], [1, 6]]), _ap(gb, [[GD, 6], [1, 6]], doff=154)
            )
            nc.vector.tensor_sub(
                aug[:, :, 0:6], t66[:], _ap(gb, [[GD, 6], [1, 6]], doff=308)
            )
            nc.vector.tensor_add(
                t6[:], _ap(gb, [[GD, 6]], doff=6), _ap(gb, [[GD, 6]], doff=160)
            )
            nc.vector.tensor_sub(
                _ap(ab, [[7, 6]], doff=6), t6[:], _ap(gb, [[GD, 6]], doff=314)
            )
            diag = _ap(ab, [[8, 6]])
            nc.vector.tensor_scalar(
                diag, diag, lamt[:, 0:1], 1.0, op0=AL.add, op1=AL.add
            )
            if iter_big:
                blk = _ap(ab, [[8, 3]], doff=3 * 7 + 3)
                nc.vector.tensor_scalar_add(blk, blk, 1e8)
                nc.vector.memset(_ap(ab, [[7, 3]], doff=3 * 7 + 6), 0.0)
            nc.vector.tensor_scalar_mul(diag, diag, 1.01)

            # ---- Gauss-Jordan, batches on partitions ----
            rp = sm_pool.tile([BPC, 1], F32, tag="rp")
            nct = sm_pool.tile([BPC, 6], F32, tag="nct")
            for k in range(6):
                nc.vector.reciprocal(rp[:], aug[:, k, k : k + 1])
                nc.vector.tensor_scalar_mul(aug[:, k, :], aug[:, k, :], rp[:, 0:1])
                nc.vector.tensor_scalar_mul(
                    nct[:], _ap(ab, [[7, 6]], doff=k), -1.0
                )
                for i in range(6):
                    if i == k:
                        continue
                    nc.vector.scalar_tensor_tensor(
                        out=aug[:, i, :],
                        in0=aug[:, k, :],
                        scalar=nct[:, i : i + 1],
                        in1=aug[:, i, :],
                        op0=AL.mult,
                        op1=AL.add,
                    )
            dpv = _ap(ab, [[7, 6]], doff=6)
            nc.gpsimd.dma_start(out=DPO[:], in_=dpv)
            dpn = sm_pool.tile([BPC, 6], F32, tag="dpn")
            nc.vector.tensor_scalar_mul(dpn[:], dpv, -1.0)
            nc.gpsimd.dma_start(out=dpb[:], in_=dpn[:])
            dpc0 = sm_pool.tile([P, BPC, 6], F32, tag="dpc0")
            nc.sync.dma_start(out=dpc0[:], in_=_dap(dpb[:], [[0, P], [6, BPC], [1, 6]]))
            dpc = sm_pool.tile([P, BPC, 6], F32, tag="dpc")
            nc.vector.tensor_copy(dpc[:], dpc0[:])

            # ---- depth pass: dd = clip(ss*(zr6 - sum_p zr_p*dp_p)) ----
            for b in range(BPC):
                zb = zr[b][:]
                nc.vector.tensor_scalar_add(
                    ddb[b][:], _ap(zb, [[8, NP]], doff=6), 0.0
                )
                for p6 in range(6):
                    nc.vector.scalar_tensor_tensor(
                        out=ddb[b][:],
                        in0=_ap(zb, [[8, NP]], doff=p6),
                        scalar=dpc[:, b, p6 : p6 + 1],
                        in1=ddb[b][:],
                        op0=AL.mult,
                        op1=AL.add,
                    )
                nc.vector.tensor_mul(
                    ddb[b][:], ddb[b][:], _ap(zb, [[8, NP]], doff=7)
                )
                nc.vector.tensor_scalar(
                    ddb[b][:], ddb[b][:], -5.0, 5.0, op0=AL.max, op1=AL.min
                )
                nc.sync.dma_start(out=DD[b], in_=ddb[b][:])
    _legalize_waits(nc)
    return nc


_BUILT = {}


def _get_nc(NP, T, iter_big):
    key = (NP, T, iter_big)
    if key not in _BUILT:
        _BUILT[key] = build_kernel(NP, T, iter_big)
    return _BUILT[key]


def _pack(r, w, J_p, J_d, lam, NP):
    Bb, Nn = r.shape[0], r.shape[1]
    conf = w[..., 0]
    nl = w[..., 1]
    sc = np.sqrt(conf)[..., None]
    cJd = conf[..., None] * J_d[..., 0]            # (B,N,2)
    H_pd = np.einsum("bncp,bnc->bnp", J_p, cJd)    # (B,N,6)
    H_dd = np.sum(J_d[..., 0] * cJd, axis=-1)      # (B,N)
    g_d = np.sum(J_d[..., 0] * conf[..., None] * r, axis=-1)
    s = 1.0 / np.maximum(H_dd + lam[:, None] + nl + np.float32(1e-4),
                         np.float32(1e-4))
    ss = np.sqrt(s)[..., None]
    Z = np.empty((Bb, Nn, W), np.float32)
    Z[..., 0:6] = sc * J_p[:, :, 0, :]
    Z[..., 6] = sc[..., 0] * r[..., 0]
    Z[..., 7:13] = sc * J_p[:, :, 1, :]
    Z[..., 13] = sc[..., 0] * r[..., 1]
    Z[..., 14:20] = ss * H_pd
    Z[..., 20] = ss[..., 0] * g_d
    Z[..., 21] = ss[..., 0]
    return np.ascontiguousarray(Z.reshape(Bb, P, NP, W))


def kernel(r, w, J_p, J_d, lmbda, iter_idx):
    r = np.asarray(r, np.float32)
    w = np.asarray(w, np.float32)
    J_p = np.asarray(J_p, np.float32)
    J_d = np.asarray(J_d, np.float32)
    lmbda = np.asarray(lmbda, np.float32)
    it = int(np.asarray(iter_idx))

    Bb, Nn = r.shape[0], r.shape[1]
    NP = Nn // P
    T = min(128, NP)
    nc = _get_nc(NP, T, it >= 2)

    lam = np.where(np.isnan(lmbda), np.float32(100.0), lmbda)
    lam = np.maximum(lam, np.float32(0.001)).reshape(Bb).astype(np.float32)

    Zp = _pack(r, w, J_p, J_d, lam, NP)
    ncores = Bb // BPC
    in_maps = [
        {"x": Zp[c * BPC : (c + 1) * BPC],
         "lam": lam[c * BPC : (c + 1) * BPC, None]}
        for c in range(ncores)
    ]
    res = run_bass_kernel_spmd(nc, in_maps, list(range(ncores)))
    dp = np.concatenate([res.results[c]["dp"] for c in range(ncores)], 0)
    dd = np.concatenate([res.results[c]["dd"] for c in range(ncores)], 0)
    dd = dd.reshape(Bb, Nn, 1)

    dp = np.clip(dp, -2.0, 2.0)
    if np.isnan(dp).any():
        dp = np.zeros_like(dp)
    if np.isnan(dd).any():
        dd = np.zeros_like(dd)
    return (dp.astype(np.float32), dd.astype(np.float32))


# revision 20
# speedup vs baseline: 1.1436x; 1.1436x over previous
"""DBA solver kernel for Trainium2 (8 NeuronCores, batch-parallel).

Host precomputes (numpy, per node):
  conf=w0, nl=w1; H_pd[p] = sum_c conf*jp[c,p]*jd[c]; H_dd = sum_c conf*jd^2
  g_d = sum_c conf*jd*r;  s = 1/max(H_dd + lam + nl + 1e-4, 1e-4)
  Z = [sqrt(conf)*jp0(6), sqrt(conf)*r0, sqrt(conf)*jp1(6), sqrt(conf)*r1,
       sqrt(s)*H_pd(6), sqrt(s)*g_d]  (21 cols),  col 21 = sqrt(s).

Device per core (2 batches), node n = p*NP + col:
  One self-Gram matmul per 128-node column accumulates G = sum Z^T Z (21x21):
    H_pp = G[0:6,0:6]+G[7:13,7:13]; g_p = G[0:6,6]+G[7:13,13]
    term = G[14:20,14:20];          g_corr = G[14:20,20]
  H_eff = H_pp + lam*I - term (+1e8 blk if iter>=2) + I; diag*=1.01
  g_eff = g_p - g_corr (rows 3: zeroed if iter>=2); 6x6 Gauss-Jordan solve.
  Depth pass from resident ZR = Z[:,14:22]:
    dd = clip(sqrt(s)*(ZR[6] - sum_p ZR[p]*dp[p]), -5, 5)
Host clips dp to [-2,2]; global-NaN -> zeros.
"""

import sys

import numpy as np

sys.path.insert(0, "/opt/trn_rl_repo")

import concourse.bass as bass  # noqa: E402
import concourse.tile as tile  # noqa: E402
from concourse import mybir  # noqa: E402
from concourse.bass_utils import run_bass_kernel_spmd  # noqa: E402

F32 = mybir.dt.float32
AL = mybir.AluOpType

# The walrus build in this container supports only ONE embedded sem-wait per
# instruction; Tile's kernel-tail drain aggregates one wait per engine/DMA
# lane (~10).  Split them across a chain of single-wait drains.
from concourse.vector_clock import ScopedClock  # noqa: E402


def _split_drain_and_barrier(self, tick_clock, wait_clock):
    drain_inst = self.nc.sync.drain()
    wait_clock.add_sem_waits(
        drain_inst.ins, ScopedClock({None: tick_clock.global_clock})
    )
    si = drain_inst.ins.sync_info
    waits = list(si.on_wait) if si and si.on_wait else []
    if len(waits) > 1:
        drain_inst.ins.sync_info = mybir.SyncInfo(
            on_wait=[waits[0]], on_update=list(si.on_update or [])
        )
        for wv in waits[1:]:
            d2 = self.nc.sync.drain()
            d2.ins.sync_info = mybir.SyncInfo(on_wait=[wv], on_update=[])
    self.nc.all_engine_barrier()
    popped = self.nc._tile_sem_poison_stack.pop()
    assert popped is self._sem_poison
    self.nc.clear_and_free_semaphores(list(self.sems.allocated().values()))
    self.nc.all_engine_barrier()


tile.TileContext._drain_and_barrier = _split_drain_and_barrier


def _legalize_waits(nc):
    """Walrus here allows one embedded wait per instruction; hoist extras
    onto preceding single-wait Drain instructions on the same engine."""
    for fn in nc.m.functions:
        for bb in fn.blocks:
            insts = list(bb.instructions)
            out = []
            changed = False
            for inst in insts:
                si = inst.sync_info
                if si is not None and si.on_wait and len(si.on_wait) > 1:
                    waits = list(si.on_wait)
                    for wv in waits[:-1]:
                        d = mybir.InstDrain(
                            name=nc.get_next_instruction_name(),
                            ins=[],
                            outs=[],
                            bass_is_fusable=False,
                        )
                        d.engine = inst.engine
                        d.sync_info = mybir.SyncInfo(on_wait=[wv], on_update=[])
                        try:
                            nc.register_instruction(d, overwrite=True)
                        except Exception:
                            pass
                        out.append(d)
                    inst.sync_info = mybir.SyncInfo(
                        on_wait=[waits[-1]], on_update=list(si.on_update or [])
                    )
                    changed = True
                out.append(inst)
            if changed:
                bb.instructions = out

NCORES = 8
BPC = 2
P = 128
W = 22  # record width: 21 Z cols + sqrt(s)
GD = 22  # gram dim (even, fp32r requirement); col 21 = sqrt(s) junk


def _ap(base, dims, doff=0):
    """View on `base` keeping its partition dim, with explicit free dims."""
    return bass.AP(
        tensor=base.tensor,
        offset=base.offset + doff,
        ap=[list(base.ap[0])] + [list(d) for d in dims],
    )


def _dap(base, dims, doff=0):
    """DRAM-side DMA AP with fully explicit dims (first dim = partitions)."""
    return bass.AP(
        tensor=base.tensor, offset=base.offset + doff, ap=[list(d) for d in dims]
    )


def build_kernel(NP, T, iter_big):
    NT = NP // T
    nc = bass.Bass()

    X = nc.declare_dram_parameter("x", [BPC, P, NP, W], mybir.dt.float32r, isOutput=False)
    LAM = nc.declare_dram_parameter("lam", [BPC, 1], F32, isOutput=False)
    DD = nc.declare_dram_parameter("dd", [BPC, P, NP], F32, isOutput=True)
    DPO = nc.declare_dram_parameter("dp", [BPC, 6], F32, isOutput=True)

    with tile.TileContext(nc) as tc:
        with (
            tc.tile_pool(name="dram", bufs=1, space="DRAM") as dram_pool,
            tc.tile_pool(name="xf", bufs=4) as xf_pool,
            tc.tile_pool(name="res", bufs=1) as res_pool,
            tc.tile_pool(name="sm", bufs=1) as sm_pool,
            tc.tile_pool(name="ps", bufs=2, space="PSUM") as ps_pool,
        ):
            scr = dram_pool.tile([BPC, GD * GD], F32, tag="scr")
            dpb = dram_pool.tile([BPC, 6], F32, tag="dpb")
            zr = [res_pool.tile([P, NP, 8], F32, tag=f"zr{b}", name=f"zr{b}")
                  for b in range(BPC)]
            ddb = [res_pool.tile([P, NP], F32, tag=f"ddb{b}", name=f"ddb{b}")
                   for b in range(BPC)]
            psum = [ps_pool.tile([GD, GD], F32, tag=f"ps{b}", name=f"ps{b}")
                    for b in range(BPC)]

            for b in range(BPC):
                for t in range(NT):
                    xf = xf_pool.tile([P, T, W], mybir.dt.float32r, tag="xf")
                    nc.sync.dma_start(out=xf[:], in_=X[b, :, t * T : (t + 1) * T, :])
                    nc.vector.tensor_copy(
                        zr[b][:, t * T : (t + 1) * T, :], xf[:, :, 14:22].bitcast(F32)
                    )
                    for j in range(T):
                        z = xf[:, j, 0:GD]
                        nc.tensor.matmul(
                            psum[b][:],
                            z,
                            z,
                            start=(t == 0 and j == 0),
                            stop=(t == NT - 1 and j == T - 1),
                        )

            # ---- extract Gram -> DRAM -> per-batch-partition gather ----
            gsb = sm_pool.tile([GD, BPC, GD], F32, tag="gsb")
            for b in range(BPC):
                nc.vector.tensor_copy(gsb[:, b, :], psum[b][:])
            nc.sync.dma_start(
                out=_dap(scr[:], [[GD, GD], [GD * GD, BPC], [1, GD]]), in_=gsb[:]
            )
            gfull = sm_pool.tile([BPC, GD * GD], F32, tag="gfull")
            nc.sync.dma_start(out=gfull[:], in_=scr[:])
            lamt0 = sm_pool.tile([BPC, 1], F32, tag="lamt0")
            nc.sync.dma_start(out=lamt0[:], in_=LAM[:, 0:1])
            lamt = sm_pool.tile([BPC, 1], F32, tag="lamt")
            nc.vector.tensor_copy(lamt[:], lamt0[:])
            gb = gfull[:]

            aug = sm_pool.tile([BPC, 6, 7], F32, tag="aug")
            t66 = sm_pool.tile([BPC, 6, 6], F32, tag="t66")
            t6 = sm_pool.tile([BPC, 6], F32, tag="t6")
            ab = aug[:]
            # diag blocks at offsets 0,154,308; g cols at 6,160,314
            nc.vector.tensor_add(
                t66[:], _ap(gb, [[GD, 6], [1, 6]]), _ap(gb, [[GD, 6], [1, 6]], doff=7 * GD + 7)
            )
            nc.vector.tensor_sub(
                aug[:, :, 0:6], t66[:], _ap(gb, [[GD, 6], [1, 6]], doff=2 * (7 * GD + 7))
            )
            nc.vector.tensor_add(
                t6[:], _ap(gb, [[GD, 6]], doff=6), _ap(gb, [[GD, 6]], doff=7 * GD + 13)
            )
            nc.vector.tensor_sub(
                _ap(ab, [[7, 6]], doff=6), t6[:], _ap(gb, [[GD, 6]], doff=2 * (7 * GD + 7) + 6)
            )
            diag = _ap(ab, [[8, 6]])
            nc.vector.tensor_scalar(
                diag, diag, lamt[:, 0:1], 1.0, op0=AL.add, op1=AL.add
            )
            if iter_big:
                blk = _ap(ab, [[8, 3]], doff=3 * 7 + 3)
                nc.vector.tensor_scalar_add(blk, blk, 1e8)
                nc.vector.memset(_ap(ab, [[7, 3]], doff=3 * 7 + 6), 0.0)
            nc.vector.tensor_scalar_mul(diag, diag, 1.01)

            # ---- Gauss-Jordan, batches on partitions ----
            rp = sm_pool.tile([BPC, 1], F32, tag="rp")
            nct = sm_pool.tile([BPC, 6], F32, tag="nct")
            for k in range(6):
                nc.vector.reciprocal(rp[:], aug[:, k, k : k + 1])
                nc.vector.tensor_scalar_mul(aug[:, k, :], aug[:, k, :], rp[:, 0:1])
                nc.vector.tensor_scalar_mul(
                    nct[:], _ap(ab, [[7, 6]], doff=k), -1.0
                )
                for i in range(6):
                    if i == k:
                        continue
                    nc.vector.scalar_tensor_tensor(
                        out=aug[:, i, :],
                        in0=aug[:, k, :],
                        scalar=nct[:, i : i + 1],
                        in1=aug[:, i, :],
                        op0=AL.mult,
                        op1=AL.add,
                    )
            dpv = _ap(ab, [[7, 6]], doff=6)
            nc.gpsimd.dma_start(out=DPO[:], in_=dpv)
            dpn = sm_pool.tile([BPC, 6], F32, tag="dpn")
            nc.vector.tensor_scalar_mul(dpn[:], dpv, -1.0)
            nc.gpsimd.dma_start(out=dpb[:], in_=dpn[:])
            dpc0 = sm_pool.tile([P, BPC, 6], F32, tag="dpc0")
            nc.sync.dma_start(out=dpc0[:], in_=_dap(dpb[:], [[0, P], [6, BPC], [1, 6]]))
            dpc = sm_pool.tile([P, BPC, 6], F32, tag="dpc")
            nc.vector.tensor_copy(dpc[:], dpc0[:])

            # ---- depth pass: dd = clip(ss*(zr6 - sum_p zr_p*dp_p)) ----
            for b in range(BPC):
                zb = zr[b][:]
                nc.vector.tensor_scalar_add(
                    ddb[b][:], _ap(zb, [[8, NP]], doff=6), 0.0
                )
                for p6 in range(6):
                    nc.vector.scalar_tensor_tensor(
                        out=ddb[b][:],
                        in0=_ap(zb, [[8, NP]], doff=p6),
                        scalar=dpc[:, b, p6 : p6 + 1],
                        in1=ddb[b][:],
                        op0=AL.mult,
                        op1=AL.add,
                    )
                nc.vector.tensor_mul(
                    ddb[b][:], ddb[b][:], _ap(zb, [[8, NP]], doff=7)
                )
                nc.vector.tensor_scalar(
                    ddb[b][:], ddb[b][:], -5.0, 5.0, op0=AL.max, op1=AL.min
                )
                nc.sync.dma_start(out=DD[b], in_=ddb[b][:])
    _legalize_waits(nc)
    return nc


_BUILT = {}


def _get_nc(NP, T, iter_big):
    key = (NP, T, iter_big)
    if key not in _BUILT:
        _BUILT[key] = build_kernel(NP, T, iter_big)
    return _BUILT[key]


def _pack(r, w, J_p, J_d, lam, NP):
    Bb, Nn = r.shape[0], r.shape[1]
    conf = w[..., 0]
    nl = w[..., 1]
    sc = np.sqrt(conf)[..., None]
    cJd = conf[..., None] * J_d[..., 0]            # (B,N,2)
    H_pd = np.einsum("bncp,bnc->bnp", J_p, cJd)    # (B,N,6)
    H_dd = np.sum(J_d[..., 0] * cJd, axis=-1)      # (B,N)
    g_d = np.sum(J_d[..., 0] * conf[..., None] * r, axis=-1)
    s = 1.0 / np.maximum(H_dd + lam[:, None] + nl + np.float32(1e-4),
                         np.float32(1e-4))
    ss = np.sqrt(s)[..., None]
    Z = np.empty((Bb, Nn, W), np.float32)
    Z[..., 0:6] = sc * J_p[:, :, 0, :]
    Z[..., 6] = sc[..., 0] * r[..., 0]
    Z[..., 7:13] = sc * J_p[:, :, 1, :]
    Z[..., 13] = sc[..., 0] * r[..., 1]
    Z[..., 14:20] = ss * H_pd
    Z[..., 20] = ss[..., 0] * g_d
    Z[..., 21] = ss[..., 0]
    return np.ascontiguousarray(Z.reshape(Bb, P, NP, W))


def kernel(r, w, J_p, J_d, lmbda, iter_idx):
    r = np.asarray(r, np.float32)
    w = np.asarray(w, np.float32)
    J_p = np.asarray(J_p, np.float32)
    J_d = np.asarray(J_d, np.float32)
    lmbda = np.asarray(lmbda, np.float32)
    it = int(np.asarray(iter_idx))

    Bb, Nn = r.shape[0], r.shape[1]
    NP = Nn // P
    T = min(128, NP)
    nc = _get_nc(NP, T, it >= 2)

    lam = np.where(np.isnan(lmbda), np.float32(100.0), lmbda)
    lam = np.maximum(lam, np.float32(0.001)).reshape(Bb).astype(np.float32)

    Zp = _pack(r, w, J_p, J_d, lam, NP)
    ncores = Bb // BPC
    in_maps = [
        {"x": Zp[c * BPC : (c + 1) * BPC],
         "lam": lam[c * BPC : (c + 1) * BPC, None]}
        for c in range(ncores)
    ]
    res = run_bass_kernel_spmd(nc, in_maps, list(range(ncores)))
    dp = np.concatenate([res.results[c]["dp"] for c in range(ncores)], 0)
    dd = np.concatenate([res.results[c]["dd"] for c in range(ncores)], 0)
    dd = dd.reshape(Bb, Nn, 1)

    dp = np.clip(dp, -2.0, 2.0)
    if np.isnan(dp).any():
        dp = np.zeros_like(dp)
    if np.isnan(dd).any():
        dd = np.zeros_like(dd)
    return (dp.astype(np.float32), dd.astype(np.float32))
